# revision 39
# baseline (speedup 1.0000x reference)
"""BertSelfAttention (disentangled seg-bias variant) on 8 Trainium2 NeuronCores.

Sharding: tensor-parallel over heads (2 heads per core), data-parallel batch
handled inside each core (both batches per core, rel_pos tiles shared).

Math per (b, h):
  q = hs @ Wq.T + bq ; k' = scale*(hs @ Wk.T) + seg_rep (host seg_rep table)
  scoresT[j,i] = sum_c k'[c,j] q[c,i]            (j on partitions)
  r1[j] = b_q_s . seg_rep[j]  -> per-partition bias of the exp (host table)
  probsT = exp(scoresT + r1[j]) * relx           (relx = exp(rel) from host;
                                   exp(a+b) = exp(a)*exp(b), shared over b)
  pv[c,i] = sum_j v[j,c] probsT[j,i] ; denom via ones-columns in the same MM
  device ships numerator + denominator rows; host does the divide.

Implementation notes:
  - ACT only does the 128 score exps (the bottleneck, ~1.07us each).
  - QK runs in fp16 with the two heads' K=64 matmuls emitted adjacently
    (PE row-group packing can run them concurrently at base partitions
    0 and 64).  fp8 q/k was tried and reverted: e4m3 quantization alone
    pushes the output error to 3.9e-2 (validated in numpy), over the gate.
  - V projection emits v directly in [j, c] layout (lhsT=hs tile,
    rhs=WvT tile) so no PE transposes / extra copies are needed.
  - PV matmuls are emitted three jt-steps late so the in-order PE queue
    never waits on the exp->mul prob chain; all prob muls run on DVE.
  - projection pieces are interleaved into the attention jt loops just
    in time; rel tiles are paced on the SP HWDGE ring by emission position.
  - the PE clock is warmed up with identity matmuls during the DMA lead-in.
"""

import numpy as np
from contextlib import ExitStack

import concourse.bass as bass
import concourse.bacc as bacc
import concourse.mybir as mybir
import concourse.tile as tile
from concourse.bass_utils import run_bass_kernel_spmd
from concourse.masks import make_identity

B, S, D, H = 2, 2048, 1024, 16
DH = D // H                      # 64
N_CORES = 8
HPC = H // N_CORES               # heads per core = 2
NKC = D // 128                   # contraction chunks = 8
NPT = S // 512                   # 512-wide position tiles = 4
NJT = S // 128                   # 128-wide j tiles = 16
NIB = S // 1024                  # 1024-wide i blocks = 2
SCALE = 1.0 / np.sqrt(DH)        # 0.125, exact in fp16

F32 = mybir.dt.float32
F16 = mybir.dt.float16


def emit_prelude(nc, pools, aps):
    (const, hspool, qkpool, vnpool, relpool, addpool, probpool,
     pspool, pvpool, ctxpool, scrpool) = pools
    (hsT, wTd, relT, segrepT, r1cin, bqc, maskT, out) = aps

    # weights are pre-transposed on host to [128, 3, NKC, 128]: one flat DMA
    w_sb = const.tile([128, 3, NKC, 128], F16, tag="w_sb")
    nc.sync.dma_start(out=w_sb, in_=wTd)
    # fold softmax scale into Wk (0.125 is exact in fp16)
    nc.vector.tensor_scalar_mul(w_sb[:, 1], w_sb[:, 1], SCALE)

    bqc_sb = const.tile([128, 1], F32, tag="bqc_sb")
    nc.sync.dma_start(out=bqc_sb, in_=bqc)
    # r1 bias columns from host, [128, (b*2+hl)*16 + jt]
    r1c = const.tile([128, B * HPC * NJT], F32, tag="r1c")
    nc.sync.dma_start(out=r1c, in_=r1cin)
    # segrep is 1MB and only needed by the first K evac (~20us in)
    segrep_sb = const.tile([128, B * S], F16, tag="segrep_sb")
    nc.sync.dma_start(out=segrep_sb, in_=segrepT)

    ident = const.tile([128, 128], F16, tag="ident")
    make_identity(nc, ident)

    # warm the PE clock (HAM releases the throttle after ~3.4us of activity)
    # while the first hsb/w DMAs are in flight
    psw = pspool.tile([128, 128], F32, tag="ps_s", name="psw")
    for _ in range(200):
        nc.tensor.matmul(psw, lhsT=ident, rhs=ident, start=True, stop=True)

    return dict(w_sb=w_sb, segrep_sb=segrep_sb, bqc_sb=bqc_sb,
                r1c=r1c, ident=ident)


def emit_body(nc, pools, aps, cst, use_mask):
    (const, hspool, qkpool, vnpool, relpool, addpool, probpool,
     pspool, pvpool, ctxpool, scrpool) = pools
    (hsT, wTd, relT, segrepT, r1cin, bqc, maskT, out) = aps
    w_sb, segrep_sb = cst["w_sb"], cst["segrep_sb"]
    bqc_sb, r1c = cst["bqc_sb"], cst["r1c"]

    hsb, qTf, kTf, vn = {}, {}, {}, {}
    relx = {}

    def hsb_dma(b, pt):
        # hsT is [B, NPT, NKC, 128, 512] on host: one instruction per pt
        if b not in hsb:
            hsb[b] = hspool.tile([128, NKC, S], F16, tag="hsb", name=f"hsb{b}")
        sl = bass.ds(pt * 512, 512)
        nc.sync.dma_start(out=hsb[b][:, :, sl],
                          in_=hsT[b, pt].rearrange("k d c -> d k c"))

    def rel_dma(ib, jts, eng=None):
        ibs = bass.ds(ib * 1024, 1024)
        for jt in jts:
            # both heads in one instruction on the SP ring; 16 tiles live
            # per ib phase, paced by emission position to arrive just in
            # time (SP-queue stalls are harmless - it runs no compute)
            r = relpool.tile([128, HPC, 1024], F16, tag="relx", name="relx",
                             bufs=16)
            (eng or nc.sync).dma_start(
                out=r,
                in_=relT[:, bass.ds(jt * 128, 128), ibs].rearrange("h j i -> j h i"))
            relx[ib, jt] = r

    def alloc_proj(b):
        if b in qTf:
            return
        qTf[b] = qkpool.tile([128, S], F16, tag="qTf", name=f"qTf{b}")
        kTf[b] = qkpool.tile([128, S], F16, tag="kTf", name=f"kTf{b}")
        vn[b] = [vnpool.tile([128, NJT, DH + 4], F16, tag=f"vn{hl}",
                             name=f"vn_b{b}h{hl}") for hl in range(HPC)]
        for hl in range(HPC):
            nc.gpsimd.memset(vn[b][hl][:, :, bass.ds(DH, 4)], 1.0)

    def qk_piece(b, which, pt):
        """project one 512-wide position chunk of q or k' into fp16"""
        alloc_proj(b)
        sl = bass.ds(pt * 512, 512)
        t = {"Q": 0, "K": 1}[which]
        ps = pspool.tile([128, 512], F32, tag="ps_s", name=f"ps{which}")
        for kk in range(NKC):
            nc.tensor.matmul(ps, lhsT=w_sb[:, t, kk], rhs=hsb[b][:, kk, sl],
                             start=(kk == 0), stop=(kk == NKC - 1))
        if which == "Q":
            nc.vector.tensor_scalar_add(qTf[b][:, sl], ps, bqc_sb)
        else:
            nc.vector.tensor_add(kTf[b][:, sl], ps,
                                 segrep_sb[:, bass.ds(b * S + pt * 512, 512)])

    def v_piece(b, jt):
        """v directly in [j, c] layout: lhsT = hs tile, rhs = WvT tile"""
        alloc_proj(b)
        jsl = bass.ds(jt * 128, 128)
        ps = pspool.tile([128, 128], F32, tag="ps_s", name="psVn")
        for kk in range(NKC):
            nc.tensor.matmul(ps, lhsT=hsb[b][:, kk, jsl], rhs=w_sb[:, 2, kk],
                             start=(kk == 0), stop=(kk == NKC - 1))
        # bv is a constant output shift (softmax weights sum to 1):
        # ctx = sum p_j (v_j + bv) / sum p_j = raw_ctx + bv -> host adds it
        for hl in range(HPC):
            nc.vector.tensor_copy(vn[b][hl][:, jt, bass.ds(0, DH)],
                                  ps[:, bass.ds(hl * DH, DH)])

    # --- attention --------------------------------------------------------
    def attn_block(ib, b, inserts):
        pv = [pvpool.tile([DH + 4, 1024], F32, tag="pv", name=f"pv{_hl}")
              for _hl in range(HPC)]
        probs = {}
        pss = {}

        def emit_qk(jt):
            psS_all = [pspool.tile([128, 1024], F32, tag="ps_s",
                                   name=f"psS{_hl}") for _hl in range(HPC)]
            for i2 in range(2):
                osl = bass.ds(i2 * 512, 512)
                for hl in range(HPC):
                    hs_ = bass.ds(hl * DH, DH)
                    # adjacent K=64 matmuls at base partitions 0/64: the PE
                    # can run them concurrently via row-group packing
                    nc.tensor.matmul(
                        psS_all[hl][:, osl],
                        lhsT=kTf[b][hs_, bass.ds(jt * 128, 128)],
                        rhs=qTf[b][hs_, bass.ds(ib * 1024 + i2 * 512, 512)],
                        start=True, stop=True)
            pss[jt] = psS_all

        def emit_exp(jt):
            if use_mask:
                msk = addpool.tile([128, 1024], F16, tag="msk")
                nc.sync.dma_start(
                    out=msk,
                    in_=maskT[b, bass.ds(jt * 128, 128), bass.ds(ib * 1024, 1024)])
            for hl in range(HPC):
                psS = pss[jt][hl]
                col = (b * HPC + hl) * NJT + jt
                bias_ap = r1c[:, col:col + 1]
                prob = probpool.tile([128, 1024], F16, tag="prob", bufs=8)
                if use_mask:
                    # relT holds plain rel in this mode; add rel + mask, then exp
                    padd = addpool.tile([128, 1024], F16, tag="padd")
                    nc.vector.tensor_add(padd, psS, relx[ib, jt][:, hl])
                    padd2 = addpool.tile([128, 1024], F16, tag="padd2")
                    nc.vector.tensor_add(padd2, padd, msk)
                    nc.scalar.activation(prob, padd2,
                                         mybir.ActivationFunctionType.Exp,
                                         bias=bias_ap, scale=1.0)
                else:
                    eqk = probpool.tile([128, 1024], F16, tag="eqk")
                    nc.scalar.activation(eqk, psS,
                                         mybir.ActivationFunctionType.Exp,
                                         bias=bias_ap, scale=1.0)
                    # all prob muls on DVE (0.59us each, 2x_1p); it has
                    # capacity and keeps the exp->prob->PV chain short
                    nc.vector.tensor_mul(prob, eqk, relx[ib, jt][:, hl])
                probs[jt, hl] = prob
            del pss[jt]

        def emit_pv(jt):
            for hl in range(HPC):
                for i2 in range(2):
                    nc.tensor.matmul(
                        pv[hl][:, bass.ds(i2 * 512, 512)],
                        lhsT=vn[b][hl][:, jt, :],
                        rhs=probs[jt, hl][:, bass.ds(i2 * 512, 512)],
                        start=(jt == 0), stop=(jt == NJT - 1))
            del probs[jt, 0], probs[jt, 1]

        # software pipeline: QK one step ahead of exp, PV three steps
        # behind, so the in-order PE queue never waits on the prob chain
        for jt in range(NJT + 2):
            if jt < NJT:
                emit_qk(jt)
            if jt >= 1 and jt - 1 < NJT:
                emit_exp(jt - 1)
            if jt >= 3:
                emit_pv(jt - 3)
            for f in inserts.get(jt, ()):
                f()
        emit_pv(NJT - 1)
        return pv

    def fin(ib, b, pv):
        # ship numerator rows + denominator row; host does the divide
        for hl in range(HPC):
            pvs = ctxpool.tile([DH + 1, 1024], F32, tag="pvs", name="pvs")
            nc.vector.tensor_copy(pvs, pv[hl][0:DH + 1, :])
            nc.sync.dma_start(
                out=out[b, hl, :, bass.ds(ib * 1024, 1024)], in_=pvs)

    # --- schedule ---------------------------------------------------------
    K = lambda b, pt: qk_piece(b, "K", pt)
    Q = lambda b, pt: qk_piece(b, "Q", pt)
    V = v_piece

    # lead-in: hsb(b0) on the SP ring; first rel tiles on the Act ring.
    # hsb(b1) and later rel tiles are paced by emission position below.
    hsb_dma(0, 0)
    rel_dma(0, [0, 1])
    hsb_dma(0, 1); hsb_dma(0, 2); hsb_dma(0, 3)
    rel_dma(0, [2, 3])
    K(0, 0); Q(0, 0); Q(0, 1)
    V(0, 0); V(0, 1)

    R = rel_dma
    ins00 = {
        0: [lambda: K(0, 1), lambda: V(0, 2), lambda: V(0, 3),
            lambda: R(0, [4])],
        1: [lambda: V(0, 4), lambda: V(0, 5), lambda: R(0, [5])],
        2: [lambda: K(0, 2), lambda: hsb_dma(1, 0), lambda: R(0, [6])],
        3: [lambda: V(0, 6), lambda: V(0, 7), lambda: hsb_dma(1, 1),
            lambda: R(0, [7, 8])],
        4: [lambda: K(0, 3), lambda: R(0, [9])],
        5: [lambda: V(0, 8), lambda: V(0, 9), lambda: hsb_dma(1, 2),
            lambda: R(0, [10, 11])],
        6: [lambda: V(0, 10), lambda: V(0, 11), lambda: R(0, [12])],
        7: [lambda: V(0, 12), lambda: V(0, 13), lambda: hsb_dma(1, 3),
            lambda: R(0, [13])],
        8: [lambda: R(0, [14]), lambda: K(1, 0)],
        9: [lambda: V(0, 14), lambda: V(0, 15), lambda: R(0, [15])],
        10: [lambda: Q(1, 0)],
        12: [lambda: Q(1, 1)],
        14: [lambda: V(1, 0), lambda: V(1, 1)],
    }
    ins01 = {
        0: [lambda: K(1, 1), lambda: R(1, [0])],
        1: [lambda: V(1, 2), lambda: V(1, 3), lambda: R(1, [1])],
        2: [lambda: K(1, 2), lambda: R(1, [2])],
        3: [lambda: V(1, 4), lambda: V(1, 5), lambda: R(1, [3])],
        4: [lambda: K(1, 3), lambda: R(1, [4])],
        5: [lambda: Q(0, 2), lambda: R(1, [5])],
        6: [lambda: V(1, 6), lambda: V(1, 7), lambda: R(1, [6])],
        7: [lambda: Q(0, 3), lambda: R(1, [7])],
        8: [lambda: V(1, 8), lambda: V(1, 9), lambda: R(1, [8])],
        9: [lambda: V(1, 10), lambda: V(1, 11), lambda: R(1, [9])],
        10: [lambda: V(1, 12), lambda: V(1, 13), lambda: R(1, [10])],
        11: [lambda: V(1, 14), lambda: V(1, 15), lambda: R(1, [11])],
        12: [lambda: R(1, [12])],
        13: [lambda: R(1, [13])],
        14: [lambda: R(1, [14, 15])],
    }
    ins10 = {
        0: [lambda: Q(1, 2)],
        2: [lambda: Q(1, 3)],
    }

    pv00 = attn_block(0, 0, ins00)
    fin(0, 0, pv00)
    pv01 = attn_block(0, 1, ins01)
    fin(0, 1, pv01)
    pv10 = attn_block(1, 0, ins10)
    fin(1, 0, pv10)
    pv11 = attn_block(1, 1, {})
    fin(1, 1, pv11)


def build_nc(use_mask=False, n_reps=1, opts=None):
    nc = bacc.Bacc("TRN2", target_bir_lowering=False, debug=False,
                   num_devices=N_CORES)
    hsT = nc.declare_dram_parameter("hsT", [B, NPT, NKC, 128, 512], F16,
                                    isOutput=False).ap()
    wTd = nc.declare_dram_parameter("wTd", [128, 3, NKC, 128], F16,
                                    isOutput=False).ap()
    relT = nc.declare_dram_parameter("relT", [HPC, S, S], F16, isOutput=False).ap()
    segrepT = nc.declare_dram_parameter("segrepT", [128, B * S], F16,
                                        isOutput=False).ap()
    r1cin = nc.declare_dram_parameter("r1c", [128, B * HPC * NJT], F32,
                                      isOutput=False).ap()
    bqc = nc.declare_dram_parameter("bqc", [128, 1], F32, isOutput=False).ap()
    maskT = (nc.declare_dram_parameter("maskT", [B, S, S], F16, isOutput=False).ap()
             if use_mask else None)
    # numerator (64 rows) + denominator (1 row) per head
    out = nc.declare_dram_parameter("out", [B, HPC, DH + 1, S], F32,
                                    isOutput=True).ap()
    aps = (hsT, wTd, relT, segrepT, r1cin, bqc, maskT, out)

    with tile.TileContext(nc) as tc, ExitStack() as ctx:
        pools = (
            ctx.enter_context(tc.tile_pool(name="const", bufs=1)),
            ctx.enter_context(tc.tile_pool(name="hspool", bufs=2)),
            ctx.enter_context(tc.tile_pool(name="qkpool", bufs=3)),
            ctx.enter_context(tc.tile_pool(name="vnpool", bufs=B)),
            ctx.enter_context(tc.tile_pool(name="relpool", bufs=8)),
            ctx.enter_context(tc.tile_pool(name="addpool", bufs=3)),
            ctx.enter_context(tc.tile_pool(name="probpool", bufs=4)),
            ctx.enter_context(tc.tile_pool(name="pspool", bufs=2, space="PSUM")),
            ctx.enter_context(tc.tile_pool(name="pvpool", bufs=2, space="PSUM")),
            ctx.enter_context(tc.tile_pool(name="ctxpool", bufs=2)),
            ctx.enter_context(tc.tile_pool(name="scrpool", bufs=1, space="DRAM")),
        )
        cst = emit_prelude(nc, pools, aps)
        if n_reps == 1:
            emit_body(nc, pools, aps, cst, use_mask)
        elif opts and opts.get("unroll"):
            for _ in range(n_reps):
                emit_body(nc, pools, aps, cst, use_mask)
        else:
            hint = (mybir.EngineType.PE, mybir.EngineType.DVE,
                    mybir.EngineType.Activation, mybir.EngineType.SP,
                    mybir.EngineType.Pool)
            with tc.For_i(0, n_reps, 1, hint_engines=hint):
                emit_body(nc, pools, aps, cst, use_mask)
    nc.compile()
    return nc


# ---------------------------------------------------------------------------
# host side
# ---------------------------------------------------------------------------

def prep_in_maps(hidden_states, attention_mask, rel_pos, seg_ids,
                 Wq, bq, Wk, Wv, bv, seg_table, b_q_s, use_mask):
    hs = np.asarray(hidden_states, np.float32)
    hsT = np.ascontiguousarray(hs.transpose(0, 2, 1)).astype(np.float16)
    # [B, D, S] -> [B, NPT, NKC, 128, 512] (pt-major contiguous chunks)
    hsT = np.ascontiguousarray(
        hsT.reshape(B, NKC, 128, NPT, 512).transpose(0, 3, 1, 2, 4))
    seg = np.asarray(seg_ids).astype(np.int64)            # [B, S]
    rel = np.asarray(rel_pos, np.float32)[0]              # [H, S, S]
    if use_mask:
        relT_all = np.ascontiguousarray(rel.transpose(0, 2, 1)).astype(np.float16)
        maskT_all = np.ascontiguousarray(
            np.asarray(attention_mask, np.float32)[:, 0].transpose(0, 2, 1)
        ).astype(np.float16)
    else:
        # relx = exp(rel): exp(qk + rel) = exp(qk) * exp(rel), shared over b
        relT_all = np.exp(rel.transpose(0, 2, 1)).astype(np.float16)
    Wq = np.asarray(Wq, np.float32); Wk = np.asarray(Wk, np.float32)
    Wv = np.asarray(Wv, np.float32)
    seg_table = np.asarray(seg_table, np.float32)
    b_q_s = np.asarray(b_q_s, np.float32)
    bq = np.asarray(bq, np.float32); bv = np.asarray(bv, np.float32)

    in_maps = []
    for c in range(N_CORES):
        hc = slice(c * HPC * DH, (c + 1) * HPC * DH)      # 128 head-columns
        wstack = np.stack([Wq[hc].T, Wk[hc].T, Wv[hc].T])  # [3, D, 128]
        wTd = np.ascontiguousarray(
            wstack.reshape(3, NKC, 128, 128).transpose(2, 0, 1, 3)
        ).astype(np.float16)                               # [128, 3, NKC, 128]
        stab_c = seg_table[:, hc]                          # [2, 128]
        # seg_rep[c, b*S+j] = seg_table[seg[b, j], hc[c]]
        segrepT = np.ascontiguousarray(
            stab_c[seg.reshape(-1)].T).astype(np.float16)  # [128, B*S]
        # r1[j] = b_q_s . seg_rep[j] per (b, hl): gamma_s[hl] = sum_c bqs*t_s
        bqs_c = b_q_s[0, c * HPC:(c + 1) * HPC, 0].reshape(128)   # [128]
        gam = (stab_c * bqs_c[None, :]).reshape(2, HPC, DH).sum(-1)  # [2, HPC]
        segf = seg.astype(np.float32).reshape(B, NJT, 128)
        r1c = np.empty((128, B * HPC * NJT), np.float32)
        for b_ in range(B):
            for hl in range(HPC):
                cols = (b_ * HPC + hl) * NJT
                r1c[:, cols:cols + NJT] = (
                    gam[0, hl] + (gam[1, hl] - gam[0, hl]) * segf[b_].T)
        m = {
            "hsT": hsT,
            "wTd": wTd,
            "relT": relT_all[c * HPC:(c + 1) * HPC],
            "segrepT": segrepT,
            "r1c": r1c,
            "bqc": bq[hc].reshape(128, 1).astype(np.float32),
        }
        if use_mask:
            m["maskT"] = maskT_all
        in_maps.append(m)
    return in_maps


def assemble_output(results, bv):
    bv = np.asarray(bv, np.float32)
    out = np.empty((B, S, D), np.float32)
    for c in range(N_CORES):
        nd = results[c]["out"]                            # [B, HPC, 65, S]
        num = nd[:, :, 0:DH, :]                           # [B, HPC, 64, S]
        den = nd[:, :, DH:DH + 1, :]                      # [B, HPC, 1, S]
        ctx = (num / den).transpose(0, 3, 1, 2)           # [B, S, HPC, 64]
        hc = slice(c * HPC * DH, (c + 1) * HPC * DH)
        out[:, :, hc] = ctx.reshape(B, S, HPC * DH) + bv[hc]
    return out


_CACHED = {}


def kernel(**inputs):
    use_mask = bool(np.any(np.asarray(inputs["attention_mask"])))
    key = ("nc", use_mask)
    if key not in _CACHED:
        _CACHED[key] = build_nc(use_mask=use_mask)
    nc = _CACHED[key]
    in_maps = prep_in_maps(use_mask=use_mask, **inputs)
    res = run_bass_kernel_spmd(nc, in_maps, list(range(N_CORES)))
    return assemble_output(res.results, inputs["bv"])
